# revision 6
# baseline (speedup 1.0000x reference)
"""Trainium2 Bass kernel for nn_CodedNet.

Reference computation (B=256, P=64, C=31):
    roll x per-channel along i, multiply by tiled sign mask, roll back,
    sum over channels.
The rolls cancel on x and only shift the mask, so the whole net collapses
to:
    out[b,i,j] = sum_c x[b,i,j,c] * mask[(i-c) % P, j]
with mask = tile(sign(w).reshape(32,32), (2,2))  (values in {-1,+1}).

Strategy: pure data parallel over batch (32 batches per core, 8 cores).

Host-side staging (dtype/layout only, no arithmetic with mixed inputs):
  - x is cast to bf16 and the channel dim padded 31 -> 32 (zeros), so
    per core x is [2048 rows=(b,i), 2048 cols=(j, c_pad)] bf16 = 8.4 MB
    (vs 16.25 MB f32): halves HBM traffic AND enables the DVE 2x perf
    mode (tensor_tensor bf16 step-1 = 2 elem/cycle; fp32 = 1).
  - The sign tile WT[p, j*32+c] = mask[(p%64 - c)%64, j] (0 in the pad
    slot) is identical for every 128-row tile, loaded once, bf16.

Per iteration, per core (16 [128, 2048] tiles, all resident), DVE rates
measured on HW at K=513 paired-slope precision:
    sync+scalar : 16 HWDGE tile loads split across BOTH HW-DGE rings
    vector      : per quad of 4 tiles,
                  - one fat tensor_mul into a SEPARATE y buffer
                    (in-place mul is ~1.8x slower: SBUF rd/wr conflict;
                    wt broadcast via a stride-0 middle dim; 2.13 elem/cyc)
                  - 5-level pairwise tensor_add tree over the padded
                    c=32 axis (2 elem/cyc; stock reduce_sum is stuck at
                    1 elem/cyc regardless of dtype = 2x slower). Last
                    level writes f32 straight into the output buffer.
                  The quad's x slots are released by the MUL (sem_g), so
                  next-iteration loads overlap the tree.
    gpsimd      : one SWDGE store of the [128, 16*64] f32 output buffer

Raw bass (no TileContext): the walrus codegen used by the axon/PJRT path
allows only one sync wait per instruction, so all waits are standalone
wait_ge ops and DMAs get dedicated semaphores. Semaphores are cleared in
a preamble (they persist across NEFF executions) behind an NRT
pseudo-barrier.

`iters > 1` repeats the pipeline with cumulative semaphore thresholds
(x reloaded from DRAM each iteration, output buffer halves alternated) -
used by bench.py to measure steady-state per-iteration HW time, since
the axon path has no NTFF profiling.
"""

import sys

sys.path.insert(0, "/opt/trn_rl_repo")

import numpy as np

B, P, C = 256, 64, 31
CP = 32                                      # padded channel dim
N_CORES = 8
ROWS_PER_CORE = (B // N_CORES) * P           # 2048
FREE = P * CP                                # 2048 (padded)
N_TILES = ROWS_PER_CORE // 128               # 16
N_QUADS = N_TILES // 4                       # 4
M = 4 * P                                    # 256 (j-groups per quad)

_CACHE = {}


def _build_program(iters: int = 1):
    """Build the Bass program (shared by all cores, SPMD)."""
    import concourse.bass as bass
    import concourse.mybir as mybir
    from contextlib import ExitStack

    bf = mybir.dt.bfloat16
    nc = bass.Bass()
    x_h = nc.declare_dram_parameter("x", [ROWS_PER_CORE, FREE], bf, isOutput=False)
    wt_h = nc.declare_dram_parameter("wt", [128, FREE], bf, isOutput=False)
    out_h = nc.declare_dram_parameter("out", [ROWS_PER_CORE, P], mybir.dt.float32, isOutput=True)

    x_t = x_h[:, :].rearrange("(n p) f -> n p f", p=128)
    # out[128t + p, j] viewed as [p, t, j] so one SBUF buffer stores all tiles
    out_t = out_h[:, :].rearrange("(n p) f -> p n f", p=128)

    ctx = ExitStack()
    with ctx:
        x_sb = ctx.enter_context(nc.sbuf_tensor([128, N_TILES * FREE], bf))
        y_sb = ctx.enter_context(nc.sbuf_tensor([128, N_TILES * FREE], bf))
        # tree level buffers: z1 [p, 16*64*16], z2 [p, 16*64*8]; levels 3/4
        # reuse z1's (fully consumed) front
        z1_sb = ctx.enter_context(nc.sbuf_tensor([128, N_TILES * P * 16], bf))
        z2_sb = ctx.enter_context(nc.sbuf_tensor([128, N_TILES * P * 8], bf))
        wt_sb = ctx.enter_context(nc.sbuf_tensor([128, FREE], bf))
        o_sb = ctx.enter_context(nc.sbuf_tensor([128, 2 * N_TILES * P], mybir.dt.float32))

        sem_w = ctx.enter_context(nc.semaphore("sem_w"))
        sem_x = [ctx.enter_context(nc.semaphore(f"sem_x{t}")) for t in range(N_TILES)]
        sem_g = [ctx.enter_context(nc.semaphore(f"sem_g{q}")) for q in range(N_QUADS)]
        sem_r = ctx.enter_context(nc.semaphore("sem_r"))
        sem_out = ctx.enter_context(nc.semaphore("sem_out"))

        # Clear all semaphores at program start (values persist across NEFF
        # executions), then an NRT pseudo-barrier so no engine can pass a
        # wait on a stale value before the clears land.
        for s in [sem_w, sem_r, sem_out, *sem_x, *sem_g]:
            nc.sync.sem_clear(s)
        nc._nrt_pseudo_barrier()

        block = ctx.enter_context(nc.Block())

        def xs(t):
            return x_sb[:, t * FREE:(t + 1) * FREE]

        @block.sync
        def _(sync):
            sync.dma_start(out=wt_sb[:, :], in_=wt_h[:, :]).then_inc(sem_w, 16)
            for k in range(iters):
                for t in range(0, N_TILES, 2):
                    if k >= 1:
                        # slot reused across iterations: previous mul done
                        sync.wait_ge(sem_g[t // 4], k)
                    sync.dma_start(out=xs(t), in_=x_t[t]).then_inc(sem_x[t], 16)

        @block.scalar
        def _(scalar):
            for k in range(iters):
                for t in range(1, N_TILES, 2):
                    if k >= 1:
                        scalar.wait_ge(sem_g[t // 4], k)
                    scalar.dma_start(out=xs(t), in_=x_t[t]).then_inc(sem_x[t], 16)

        @block.vector
        def _(vector):
            vector.wait_ge(sem_w, 16)
            for k in range(iters):
                # 4 quad muls (x slots released per quad), then 5 fat tree
                # levels across all 16 tiles (fewer, fatter DVE ops)
                for q in range(N_QUADS):
                    vector.wait_ge(sem_x[4 * q + 2], 16 * (k + 1))
                    vector.wait_ge(sem_x[4 * q + 3], 16 * (k + 1))
                    quad = x_sb[:, 4 * q * FREE:(4 * q + 4) * FREE]
                    wt_b = wt_sb[:, :].unsqueeze(1).broadcast_to([128, 4, FREE])
                    nc.vector.tensor_mul(
                        y_sb[:, 4 * q * FREE:(4 * q + 4) * FREE]
                            .rearrange("p (n f) -> p n f", n=4),
                        quad.rearrange("p (n f) -> p n f", n=4),
                        wt_b,
                    ).then_inc(sem_g[q], 1)
                # pairwise tree over c: 32 -> 16 -> 8 -> 4 -> 2 -> 1
                y = y_sb[:, :].rearrange("p (m c) -> p m c", c=CP)
                a1 = z1_sb[:, :].rearrange("p (m c) -> p m c", c=16)
                nc.vector.tensor_add(a1, y[:, :, 0:16], y[:, :, 16:32])
                a2 = z2_sb[:, :].rearrange("p (m c) -> p m c", c=8)
                nc.vector.tensor_add(a2, a1[:, :, 0:8], a1[:, :, 8:16])
                a3 = z1_sb[:, :N_TILES * P * 4].rearrange("p (m c) -> p m c", c=4)
                nc.vector.tensor_add(a3, a2[:, :, 0:4], a2[:, :, 4:8])
                a4 = z1_sb[:, N_TILES * P * 4:N_TILES * P * 6] \
                    .rearrange("p (m c) -> p m c", c=2)
                nc.vector.tensor_add(a4, a3[:, :, 0:2], a3[:, :, 2:4])
                if k >= 2:
                    # o_sb half reuse: store of iteration k-2 done
                    vector.wait_ge(sem_out, 16 * (k - 1))
                off = (k % 2) * N_TILES * P
                nc.vector.tensor_add(
                    o_sb[:, off:off + N_TILES * P]
                        .rearrange("p (m c) -> p m c", c=1),
                    a4[:, :, 0:1], a4[:, :, 1:2],
                ).then_inc(sem_r, 1)

        @block.gpsimd
        def _(gpsimd):
            for k in range(iters):
                gpsimd.wait_ge(sem_r, k + 1)
                if k >= 1:
                    gpsimd.wait_ge(sem_out, 16 * k)
                gpsimd.dma_start(
                    out=out_t,
                    in_=o_sb[:, (k % 2) * N_TILES * P:((k % 2) + 1) * N_TILES * P]
                        .rearrange("p (n f) -> p n f", f=P),
                ).then_inc(sem_out, 16)
            gpsimd.wait_ge(sem_out, 16 * iters)
    return nc


def _get_program(iters: int = 1):
    key = ("nc", iters)
    if key not in _CACHE:
        _CACHE[key] = _build_program(iters)
    return _CACHE[key]


def _sign_tile(w: np.ndarray) -> np.ndarray:
    """WT[p, j*32+c] = mask[(p%64 - c)%64, j] for c<31, 0 in the pad slot (bf16)."""
    import ml_dtypes

    mask = np.tile(np.sign(w.astype(np.float32)).reshape(32, 32), (2, 2))  # [64, 64] = (r, j)
    i_idx = np.arange(128) % P
    c_idx = np.arange(C)
    j_idx = np.arange(P)
    wt = np.zeros((128, P, CP), dtype=np.float32)
    wt[:, :, :C] = mask[
        (i_idx[:, None, None] - c_idx[None, None, :]) % P, j_idx[None, :, None]
    ]
    return np.ascontiguousarray(wt.reshape(128, FREE)).astype(ml_dtypes.bfloat16)


def _stage_x(x: np.ndarray) -> np.ndarray:
    """[B,P,P,C] f32 -> [B*P, P*CP] bf16 with the channel dim zero-padded."""
    import ml_dtypes

    xp = np.zeros((B * P, P, CP), dtype=ml_dtypes.bfloat16)
    xp[:, :, :C] = x.reshape(B * P, P, C).astype(ml_dtypes.bfloat16)
    return xp.reshape(B * P, FREE)


def kernel(x: np.ndarray, w: np.ndarray) -> np.ndarray:
    from concourse.bass_utils import run_bass_kernel_spmd

    nc = _get_program()
    wt = _sign_tile(w)
    x2 = _stage_x(np.asarray(x, dtype=np.float32))
    in_maps = [
        {"x": x2[k * ROWS_PER_CORE:(k + 1) * ROWS_PER_CORE], "wt": wt}
        for k in range(N_CORES)
    ]
    res = run_bass_kernel_spmd(nc, in_maps, list(range(N_CORES)))
    out = np.concatenate([res.results[k]["out"] for k in range(N_CORES)], axis=0)
    return out.reshape(B, P, P)


# revision 7
# speedup vs baseline: 1.0282x; 1.0282x over previous
"""Trainium2 Bass kernel for nn_CodedNet.

Reference computation (B=256, P=64, C=31):
    roll x per-channel along i, multiply by tiled sign mask, roll back,
    sum over channels.
The rolls cancel on x and only shift the mask, so the whole net collapses
to:
    out[b,i,j] = sum_c x[b,i,j,c] * mask[(i-c) % P, j]
with mask = tile(sign(w).reshape(32,32), (2,2))  (values in {-1,+1}).

Strategy: pure data parallel over batch (32 batches per core, 8 cores).

Host-side staging (dtype/layout only, no arithmetic with mixed inputs):
  - x is cast to bf16 and the channel dim padded 31 -> 32 (zeros), so
    per core x is [2048 rows=(b,i), 2048 cols=(j, c_pad)] bf16 = 8.4 MB
    (vs 16.25 MB f32): halves HBM traffic AND enables the DVE 2x perf
    mode (tensor_tensor bf16 step-1 = 2 elem/cycle; fp32 = 1).
  - The sign tile WT[p, j*32+c] = mask[(p%64 - c)%64, j] (0 in the pad
    slot) is identical for every 128-row tile, loaded once, bf16.

Per iteration, per core (16 [128, 2048] tiles, all resident), DVE rates
measured on HW at K=513 paired-slope precision:
    sync+scalar : 16 HWDGE tile loads split across BOTH HW-DGE rings
    vector      : per quad of 4 tiles,
                  - one fat tensor_mul into a SEPARATE y buffer
                    (in-place mul is ~1.8x slower: SBUF rd/wr conflict;
                    wt broadcast via a stride-0 middle dim; 2.13 elem/cyc)
                  - 5-level pairwise tensor_add tree over the padded
                    c=32 axis (2 elem/cyc; stock reduce_sum is stuck at
                    1 elem/cyc regardless of dtype = 2x slower). Last
                    level writes f32 straight into the output buffer.
                  The quad's x slots are released by the MUL (sem_g), so
                  next-iteration loads overlap the tree.
    gpsimd      : one SWDGE store of the [128, 16*64] f32 output buffer

Raw bass (no TileContext): the walrus codegen used by the axon/PJRT path
allows only one sync wait per instruction, so all waits are standalone
wait_ge ops and DMAs get dedicated semaphores. Semaphores are cleared in
a preamble (they persist across NEFF executions) behind an NRT
pseudo-barrier.

`iters > 1` repeats the pipeline with cumulative semaphore thresholds
(x reloaded from DRAM each iteration, output buffer halves alternated) -
used by bench.py to measure steady-state per-iteration HW time, since
the axon path has no NTFF profiling.
"""

import sys

sys.path.insert(0, "/opt/trn_rl_repo")

import numpy as np

B, P, C = 256, 64, 31
CP = 32                                      # padded channel dim
N_CORES = 8
ROWS_PER_CORE = (B // N_CORES) * P           # 2048
FREE = P * CP                                # 2048 (padded)
N_TILES = ROWS_PER_CORE // 128               # 16
N_QUADS = N_TILES // 4                       # 4

_CACHE = {}


def _build_program(iters: int = 1):
    """Build the Bass program (shared by all cores, SPMD)."""
    import concourse.bass as bass
    import concourse.mybir as mybir
    from contextlib import ExitStack

    bf = mybir.dt.bfloat16
    nc = bass.Bass()
    x_h = nc.declare_dram_parameter("x", [ROWS_PER_CORE, FREE], bf, isOutput=False)
    wt_h = nc.declare_dram_parameter("wt", [128, FREE], bf, isOutput=False)
    out_h = nc.declare_dram_parameter("out", [ROWS_PER_CORE, P], mybir.dt.float32, isOutput=True)

    x_t = x_h[:, :].rearrange("(n p) f -> n p f", p=128)
    # out[128t + p, j] viewed as [p, t, j] so one SBUF buffer stores all tiles
    out_t = out_h[:, :].rearrange("(n p) f -> p n f", p=128)

    ctx = ExitStack()
    with ctx:
        x_sb = ctx.enter_context(nc.sbuf_tensor([128, N_TILES * FREE], bf))
        y_sb = ctx.enter_context(nc.sbuf_tensor([128, N_TILES * FREE], bf))
        # tree level buffers: z1 [p, 16*64*16], z2 [p, 16*64*8]; levels 3/4
        # reuse z1's (fully consumed) front
        z1_sb = ctx.enter_context(nc.sbuf_tensor([128, N_TILES * P * 16], bf))
        z2_sb = ctx.enter_context(nc.sbuf_tensor([128, N_TILES * P * 8], bf))
        wt_sb = ctx.enter_context(nc.sbuf_tensor([128, FREE], bf))
        o_sb = ctx.enter_context(nc.sbuf_tensor([128, 2 * N_TILES * P], mybir.dt.float32))

        sem_w = ctx.enter_context(nc.semaphore("sem_w"))
        sem_x = [ctx.enter_context(nc.semaphore(f"sem_x{t}")) for t in range(N_TILES)]
        sem_g = [ctx.enter_context(nc.semaphore(f"sem_g{q}")) for q in range(N_QUADS)]
        sem_r = ctx.enter_context(nc.semaphore("sem_r"))
        sem_out = ctx.enter_context(nc.semaphore("sem_out"))

        # Clear all semaphores at program start (values persist across NEFF
        # executions), then an NRT pseudo-barrier so no engine can pass a
        # wait on a stale value before the clears land.
        for s in [sem_w, sem_r, sem_out, *sem_x, *sem_g]:
            nc.sync.sem_clear(s)
        nc._nrt_pseudo_barrier()

        block = ctx.enter_context(nc.Block())

        def xs(t):
            return x_sb[:, t * FREE:(t + 1) * FREE]

        @block.sync
        def _(sync):
            sync.dma_start(out=wt_sb[:, :], in_=wt_h[:, :]).then_inc(sem_w, 16)
            for k in range(iters):
                for t in range(0, N_TILES, 2):
                    if k >= 1:
                        # slot reused across iterations: previous mul done
                        sync.wait_ge(sem_g[t // 4], k)
                    sync.dma_start(out=xs(t), in_=x_t[t]).then_inc(sem_x[t], 16)

        @block.scalar
        def _(scalar):
            for k in range(iters):
                for t in range(1, N_TILES, 2):
                    if k >= 1:
                        scalar.wait_ge(sem_g[t // 4], k)
                    scalar.dma_start(out=xs(t), in_=x_t[t]).then_inc(sem_x[t], 16)

        @block.vector
        def _(vector):
            vector.wait_ge(sem_w, 16)
            for k in range(iters):
                # 4 quad muls (x slots released per quad), then 5 fat tree
                # levels across all 16 tiles (fewer, fatter DVE ops)
                for q in range(N_QUADS):
                    vector.wait_ge(sem_x[4 * q + 2], 16 * (k + 1))
                    vector.wait_ge(sem_x[4 * q + 3], 16 * (k + 1))
                    quad = x_sb[:, 4 * q * FREE:(4 * q + 4) * FREE]
                    wt_b = wt_sb[:, :].unsqueeze(1).broadcast_to([128, 4, FREE])
                    nc.vector.tensor_mul(
                        y_sb[:, 4 * q * FREE:(4 * q + 4) * FREE]
                            .rearrange("p (n f) -> p n f", n=4),
                        quad.rearrange("p (n f) -> p n f", n=4),
                        wt_b,
                    ).then_inc(sem_g[q], 1)
                # pairwise tree over c: 32 -> 16 -> 8 -> 4 -> 2 -> 1
                y = y_sb[:, :].rearrange("p (m c) -> p m c", c=CP)
                a1 = z1_sb[:, :].rearrange("p (m c) -> p m c", c=16)
                nc.vector.tensor_add(a1, y[:, :, 0:16], y[:, :, 16:32])
                a2 = z2_sb[:, :].rearrange("p (m c) -> p m c", c=8)
                nc.vector.tensor_add(a2, a1[:, :, 0:8], a1[:, :, 8:16])
                a3 = z1_sb[:, :N_TILES * P * 4].rearrange("p (m c) -> p m c", c=4)
                nc.vector.tensor_add(a3, a2[:, :, 0:4], a2[:, :, 4:8])
                a4 = z1_sb[:, N_TILES * P * 4:N_TILES * P * 6] \
                    .rearrange("p (m c) -> p m c", c=2)
                nc.vector.tensor_add(a4, a3[:, :, 0:2], a3[:, :, 2:4])
                if k >= 2:
                    # o_sb half reuse: store of iteration k-2 done
                    vector.wait_ge(sem_out, 16 * (k - 1))
                off = (k % 2) * N_TILES * P
                nc.vector.tensor_add(
                    o_sb[:, off:off + N_TILES * P]
                        .rearrange("p (m c) -> p m c", c=1),
                    a4[:, :, 0:1], a4[:, :, 1:2],
                ).then_inc(sem_r, 1)

        @block.gpsimd
        def _(gpsimd):
            for k in range(iters):
                gpsimd.wait_ge(sem_r, k + 1)
                if k >= 1:
                    gpsimd.wait_ge(sem_out, 16 * k)
                gpsimd.dma_start(
                    out=out_t,
                    in_=o_sb[:, (k % 2) * N_TILES * P:((k % 2) + 1) * N_TILES * P]
                        .rearrange("p (n f) -> p n f", f=P),
                ).then_inc(sem_out, 16)
            gpsimd.wait_ge(sem_out, 16 * iters)
    return nc


def _get_program(iters: int = 1):
    key = ("nc", iters)
    if key not in _CACHE:
        _CACHE[key] = _build_program(iters)
    return _CACHE[key]


def _sign_tile(w: np.ndarray) -> np.ndarray:
    """WT[p, j*32+c] = mask[(p%64 - c)%64, j] for c<31, 0 in the pad slot (bf16)."""
    import ml_dtypes

    mask = np.tile(np.sign(w.astype(np.float32)).reshape(32, 32), (2, 2))  # [64, 64] = (r, j)
    i_idx = np.arange(128) % P
    c_idx = np.arange(C)
    j_idx = np.arange(P)
    wt = np.zeros((128, P, CP), dtype=np.float32)
    wt[:, :, :C] = mask[
        (i_idx[:, None, None] - c_idx[None, None, :]) % P, j_idx[None, :, None]
    ]
    return np.ascontiguousarray(wt.reshape(128, FREE)).astype(ml_dtypes.bfloat16)


def _stage_x(x: np.ndarray) -> np.ndarray:
    """[B,P,P,C] f32 -> [B*P, P*CP] bf16 with the channel dim zero-padded."""
    import ml_dtypes

    xp = np.zeros((B * P, P, CP), dtype=ml_dtypes.bfloat16)
    xp[:, :, :C] = x.reshape(B * P, P, C).astype(ml_dtypes.bfloat16)
    return xp.reshape(B * P, FREE)


def kernel(x: np.ndarray, w: np.ndarray) -> np.ndarray:
    from concourse.bass_utils import run_bass_kernel_spmd

    nc = _get_program()
    wt = _sign_tile(w)
    x2 = _stage_x(np.asarray(x, dtype=np.float32))
    in_maps = [
        {"x": x2[k * ROWS_PER_CORE:(k + 1) * ROWS_PER_CORE], "wt": wt}
        for k in range(N_CORES)
    ]
    res = run_bass_kernel_spmd(nc, in_maps, list(range(N_CORES)))
    out = np.concatenate([res.results[k]["out"] for k in range(N_CORES)], axis=0)
    return out.reshape(B, P, P)
